# revision 22
# baseline (speedup 1.0000x reference)
"""Trainium2 Bass kernel for nn_Attention_65609920414302 (sparse multi-branch attention).

Sharding: 64 total heads (4 branches x 16 sub-heads) split as 8 heads per core
(core c = branch c//2, base-head half c%2). Each core computes Q/K/V projections
for its heads, RoPE, causal thresholded-softplus attention, and a partial W_O
matmul; the host sums the 8 partial outputs.

Math rescaling used on device (S = pi/sqrt(3)):
  reference w_sig = w*sigmoid(S*w) with w = softplus(scores*m), thresholded at sink.
  device   W = silu(S*w) = S*w_sig, thresholded at S*sink,
  probs    = W / (sum_s W + S*(sink+1e-6)),  sink term = S*sink / (...).
The S factors cancel exactly. softplus is composed as ln(1 + exp(x)) since this
toolchain has no softplus ACT table.

v2 pipeline (vs the phase-serial baseline):
  - per-group software pipeline: K(g) -> m(g) -> Q(g) -> scores(g)/exp(g) so the
    ACT engine starts ~14us in and runs a dense exp/ln/silu stream.
  - the per-key scale m = 1/(8*sqrt(key_self)) is premultiplied into krope
    (Pool partition_broadcast + DVE mult) so exp needs no per-partition scale,
    allowing score blocks packed into [128,1024] PSUM tiles (5 exps/head).
  - m itself is sqrt(r)/8 computed as exp(0.5*ln(r) - ln(8)) on the SAME
    ln/exp ACT table (no sqrt-table thrash).
  - Q/K biases folded into the PSUM->SBUF copy (per-partition tensor_scalar).
  - W_O runs in 3 rounds of 8 [128,1024] units: ct0+ct1 (PSUM pair) -> y_acc,
    ct2 -> add, ct3 (K=64 halves) -> final add -> DMA.
  - work spread across engines: combines/t2/broadcasts/V-copies/masks on Pool,
    copies/thresholds/normalize on DVE, exp/ln/silu/m on ACT.
"""

import math
import os
import numpy as np

D_MODEL = 1024
N_HEAD = 16
N_BR = 4
DH = 64
H_TOT = 64
T = 1024
S = math.pi / math.sqrt(3.0)
N_CORES = 8
HPC = 8          # heads per core
KT = 8           # C // 128 contraction tiles
W_COLS = 4608    # sum of causal-trapezoid block widths

# packed block order inside wbuf: pairs that sum to 1024 columns share a PSUM
# tile: (0), (1,7), (2,6), (3,5), (4)
BLK_ORDER = [0, 1, 7, 2, 6, 3, 5, 4]
BLK_LEN = {i: T - 128 * i for i in range(8)}
OFF = {}
_cur = 0
for _b in BLK_ORDER:
    OFF[_b] = _cur
    _cur += BLK_LEN[_b]
assert _cur == W_COLS
TILE_GROUPS = [(0,), (1, 7), (2, 6), (3, 5), (4,)]

_NC_CACHE = [None]
LAST_RESULT = [None]  # stash for test harness (exec_time_ns etc.)


def _build_nc():
    import concourse.bass as bass
    from concourse import bacc
    import concourse.mybir as mybir
    import concourse.tile as tile
    from concourse.tile import add_dep_helper

    F32 = mybir.dt.float32
    F32R = mybir.dt.float32r
    F16 = mybir.dt.float16
    AF = mybir.ActivationFunctionType
    ALU = mybir.AluOpType

    nc = bacc.Bacc(None, target_bir_lowering=False, debug=False)

    # ---- DRAM parameters (per-core data; same program on all cores) ----
    XT = nc.declare_dram_parameter("XT", [D_MODEL, T], F16, isOutput=False)
    WQ = nc.declare_dram_parameter("WQ", [D_MODEL, 512], F16, isOutput=False)
    WK = nc.declare_dram_parameter("WK", [D_MODEL, 512], F16, isOutput=False)
    WV = nc.declare_dram_parameter("WV", [D_MODEL, 512], F16, isOutput=False)
    BQT = nc.declare_dram_parameter("BQT", [128, 4], F32, isOutput=False)
    BKT = nc.declare_dram_parameter("BKT", [128, 4], F32, isOutput=False)
    BV = nc.declare_dram_parameter("BV", [1, 512], F16, isOutput=False)
    WO = nc.declare_dram_parameter("WO", [512, D_MODEL], F32R, isOutput=False)
    COS = nc.declare_dram_parameter("COS", [128, T], F16, isOutput=False)
    SIN = nc.declare_dram_parameter("SIN", [128, T], F16, isOutput=False)
    PSW = nc.declare_dram_parameter("PSW", [128, 128], F16, isOutput=False)
    SEL2 = nc.declare_dram_parameter("SEL2", [128, 4, 2], F16, isOutput=False)
    BCH = nc.declare_dram_parameter("BCH", [2, 128], F16, isOutput=False)
    THR = nc.declare_dram_parameter("THR", [128, 8], F32, isOutput=False)
    TB = nc.declare_dram_parameter("TB", [1, 8], F32, isOutput=False)
    VNS = nc.declare_dram_parameter("VNS", [64, 8], F32, isOutput=False)
    ONES = nc.declare_dram_parameter("ONES", [1, 512], F16, isOutput=False)
    YT = nc.declare_dram_parameter("YT", [D_MODEL, T], F32, isOutput=True)
    dbg = bool(os.environ.get("KDEBUG"))
    if dbg:
        DKR = nc.declare_dram_parameter("DKR", [128, 4, T], F16, isOutput=True)
        DQR = nc.declare_dram_parameter("DQR", [128, 4, T], F16, isOutput=True)
        DWB = nc.declare_dram_parameter("DWB", [128, 2, W_COLS], F16, isOutput=True)
        DCTX = nc.declare_dram_parameter("DCTX", [128, 4, T], F32R, isOutput=True)

    with tile.TileContext(nc) as tc:
        pc = tc.alloc_tile_pool(name="const", bufs=1)
        pk = tc.alloc_tile_pool(name="keep", bufs=1)
        tr = tc.alloc_tile_pool(name="trans", bufs=2)
        pw = tc.alloc_tile_pool(name="wbuf", bufs=3)
        pa = tc.alloc_tile_pool(name="psall", bufs=1, space="PSUM")

        # ---- constants ----
        cos_sb = pc.tile([128, T], F16)
        sin_sb = pc.tile([128, T], F16)
        psw_sb = pc.tile([128, 128], F16)
        sel2_sb = pc.tile([128, 4, 2], F16)
        bch_sb = pc.tile([2, 128], F16)
        thr_sb = pc.tile([128, 8], F32)
        tb_sb = pc.tile([1, 8], F32)
        vns_sb = pc.tile([64, 8], F32)
        ones_r = pc.tile([1, 512], F16)
        bqt_sb = pc.tile([128, 4], F32)
        bkt_sb = pc.tile([128, 4], F32)

        nc.sync.dma_start(out=ones_r, in_=ONES.ap())
        nc.sync.dma_start(out=psw_sb, in_=PSW.ap())
        nc.sync.dma_start(out=sel2_sb, in_=SEL2.ap())
        nc.sync.dma_start(out=bch_sb, in_=BCH.ap())
        nc.sync.dma_start(out=cos_sb, in_=COS.ap())
        nc.sync.dma_start(out=sin_sb, in_=SIN.ap())
        nc.sync.dma_start(out=bkt_sb, in_=BKT.ap())
        nc.sync.dma_start(out=bqt_sb, in_=BQT.ap())

        # ---- weights ----
        xt = pk.tile([128, KT, T], F16)
        wq = pk.tile([128, KT, 4, 128], F16)
        wk = pk.tile([128, KT, 4, 128], F16)
        wv = pk.tile([128, KT, 512], F16)
        bv = pc.tile([1, 512], F16)
        xt_src = XT.ap().rearrange("(kt p) t -> p kt t", p=128)
        wk_src = WK.ap().rearrange("(kt p) (mt m) -> p kt mt m", p=128, m=128)
        for kt in range(KT):
            nc.sync.dma_start(out=xt[:, kt, :], in_=xt_src[:, kt, :])
            nc.sync.dma_start(out=wk[:, kt, :, :], in_=wk_src[:, kt, :, :])
        nc.sync.dma_start(
            out=wq, in_=WQ.ap().rearrange("(kt p) (mt m) -> p kt mt m", p=128, m=128)
        )
        nc.sync.dma_start(out=thr_sb, in_=THR.ap())
        nc.sync.dma_start(out=tb_sb, in_=TB.ap())
        nc.sync.dma_start(out=vns_sb, in_=VNS.ap())
        nc.sync.dma_start(out=bv, in_=BV.ap())
        nc.sync.dma_start(out=wv, in_=WV.ap().rearrange("(kt p) v -> p kt v", p=128))
        wo = pk.tile([128, 4, 8, 128], F32R)
        nc.sync.dma_start(
            out=wo, in_=WO.ap().rearrange("(ct p) (mt m) -> p ct mt m", p=128, m=128)
        )

        qrope = pk.tile([128, 4, T], F16)
        krope = pk.tile([128, 4, T], F16)
        vstore = pk.tile([128, 8, HPC, 65], F16)
        ctx = pk.tile([128, 4, T], F32R)
        y_acc = pk.tile([128, 8, T], F16)
        nc.vector.memset(vstore[:, :, :, 64:65], 1.0)

        # warm up the PE clock (HAM) with dummy matmuls while DMAs stream in
        wu_ps = pa.tile([1, 512], F32, tag="pvproj", bufs=2)
        for _ in range(16):
            nc.tensor.matmul(wu_ps, ones_r[0:1, 0:1], ones_r, start=True,
                             stop=True)

        def warm_pe(n):
            wu = pa.tile([1, 512], F32, tag="pvproj", bufs=2)
            for _ in range(n):
                nc.tensor.matmul(wu, ones_r[0:1, 0:1], ones_r, start=True,
                                 stop=True)

        # ---------- emission helpers ----------

        def proj_chain(w_t, g):
            """X @ W slice for group g -> [128, T] PSUM tile."""
            ps = pa.tile([128, T], F32, tag="pvproj", bufs=2)
            for th in range(2):
                sl = slice(512 * th, 512 * (th + 1))
                for kt in range(KT):
                    nc.tensor.matmul(
                        ps[:, sl], w_t[:, kt, g, :], xt[:, kt, sl],
                        start=(kt == 0), stop=(kt == KT - 1),
                    )
            return ps

        def rope_common(ps, bias_t, g, out_t):
            """bias-add copy, swap matmul, cos/sin muls, combine -> out_t[:, g, :].
            Returns the f16 pre-rope projection (for key_self)."""
            qsb = tr.tile([128, T], F16, tag="qsb")
            nc.vector.tensor_scalar_add(qsb, ps, bias_t[:, g:g + 1])
            sw = pa.tile([128, T], F32, tag="pvproj", bufs=2)
            for th in range(2):
                sl = slice(512 * th, 512 * (th + 1))
                nc.tensor.matmul(sw[:, sl], psw_sb, qsb[:, sl], start=True,
                                 stop=True)
            t1 = tr.tile([128, T], F16, tag="t1", bufs=1)
            nc.vector.tensor_tensor(t1, qsb, cos_sb, op=ALU.mult)
            t2 = tr.tile([128, T], F16, tag="t2")
            nc.vector.tensor_tensor(t2, sw, sin_sb, op=ALU.mult)
            nc.gpsimd.tensor_tensor(out_t[:, g, :], t1, t2, op=ALU.add)
            return qsb

        def finish_k_phase(g, ps):
            """key_self -> m chain + K rope + krope scaling."""
            ksb = rope_common(ps, bkt_sb, g, krope)
            # key_self from the pre-RoPE projection (rotation-invariant)
            k2 = tr.tile([128, T], F16, tag="k2", bufs=1)
            nc.vector.tensor_tensor(k2, ksb, ksb, op=ALU.mult)
            ks = pa.tile([2, T], F32, tag="pvproj", bufs=2)
            for th in range(2):
                sl = slice(512 * th, 512 * (th + 1))
                nc.tensor.matmul(ks[:, sl], sel2_sb[:, g, :], k2[:, sl],
                                 start=True, stop=True)
            # m = ATTNSCALE/sqrt(key_self) = exp(0.5*ln(recip(ks)) - ln 8),
            # via the already-loaded ln/exp table (no sqrt table load)
            mx = tr.tile([2, T], F32, tag="mx", bufs=1)
            nc.vector.tensor_scalar_max(mx, ks, 1e-6)
            nc.vector.reciprocal_approx_fast(mx, mx)
            # m = sqrt(r/64) = exp(0.5*ln(r/64)), on the shared ln/exp table
            nc.scalar.activation(mx, mx, AF.Ln, scale=1.0 / 64.0)
            m2 = tr.tile([2, T], F16, tag="m2")
            nc.scalar.activation(m2, mx, AF.Exp, scale=0.5)
            # broadcast m to both 64-row halves via a tiny PE outer product,
            # then fold into krope with a single full-width multiply
            mb = pa.tile([128, T], F32, tag="pvproj", bufs=2)
            for th in range(2):
                sl = slice(512 * th, 512 * (th + 1))
                nc.tensor.matmul(mb[:, sl], bch_sb, m2[:, sl], start=True,
                                 stop=True)
            nc.vector.tensor_tensor(krope[:, g, :], krope[:, g, :], mb,
                                    op=ALU.mult)

        def emit_q_phase(g):
            ps = proj_chain(wq, g)
            rope_common(ps, bqt_sb, g, qrope)

        def score_tile_units(g):
            """Yield per-PSUM-tile closures for wave g's scores+exp; each unit
            emits matmuls for one (head, tile-group) and its exp."""
            wbuf = wbuf_of[g]
            units = []
            for ti, blocks in enumerate(TILE_GROUPS):
                for u in range(2):
                    r0 = 64 * u
                    units.append((ti, blocks, u, r0))

            def emit_unit(idx):
                ti, blocks, u, r0 = units[idx]
                ncols = sum(BLK_LEN[b] for b in blocks)
                woff = OFF[blocks[0]]
                ps_s = pa.tile([128, T], F32, tag="scores", bufs=2)
                p0 = 0
                for b in blocks:
                    L = BLK_LEN[b]
                    t0 = 128 * b
                    c = p0
                    while c < p0 + L:
                        nxt = min((c // 512 + 1) * 512, p0 + L)
                        nc.tensor.matmul(
                            ps_s[:, c:nxt],
                            krope[r0:r0 + 64, g, t0:t0 + 128],
                            qrope[r0:r0 + 64, g, t0 + (c - p0):t0 + (nxt - p0)],
                            start=True, stop=True,
                        )
                        c = nxt
                    p0 += L
                e = nc.scalar.activation(
                    wbuf[:, u, woff:woff + ncols], ps_s[:, 0:ncols], AF.Exp
                )
                exp_insts_of[g].append(e)
                for si in silu_of.get(g - 1, []):
                    add_dep_helper(e.ins, si.ins, sync=False,
                                   reason="act table phase order")

            return [lambda i=i: emit_unit(i) for i in range(len(units))]

        def emit_mask(g, b):
            """Zero the upper-triangular part of diagonal block b for both
            heads of wave g (keys > query)."""
            wbuf = wbuf_of[g]
            o = OFF[b]
            for u in range(2):
                nc.gpsimd.affine_select(
                    out=wbuf[:, u, o:o + 128], in_=wbuf[:, u, o:o + 128],
                    compare_op=ALU.is_ge, fill=0.0, base=0,
                    pattern=[[1, 128]], channel_multiplier=-1,
                )

        def emit_ln_silu(g):
            wbuf = wbuf_of[g]
            ln = nc.scalar.activation(wbuf[:, :, :], wbuf[:, :, :], AF.Ln,
                                      bias=1.0)
            for e in exp_insts_of[g]:
                add_dep_helper(ln.ins, e.ins, sync=False,
                               reason="act table phase order")
            silu_of[g] = []
            for u in range(2):
                si = nc.scalar.activation(wbuf[:, u, :], wbuf[:, u, :], AF.Silu,
                                          scale=S)
                add_dep_helper(si.ins, ln.ins, sync=False,
                               reason="act table phase order")
                silu_of[g].append(si)

        def emit_head_tail(g, u):
            """threshold + PV + normalize for head h = 2g+u -> ctx."""
            h = 2 * g + u
            r0 = 64 * u
            wbuf = wbuf_of[g]
            nc.vector.scalar_tensor_tensor(
                out=wbuf[:, u, :], in0=wbuf[:, u, :],
                scalar=thr_sb[:, h:h + 1], in1=wbuf[:, u, :],
                op0=ALU.is_ge, op1=ALU.mult,
            )
            ps_pv = pa.tile([65, T], F32, tag="pvproj", bufs=2)
            for i in range(8):
                t0 = 128 * i
                o = OFF[i]
                if t0 < 512:
                    chunks = [(t0, 512, 3), (512, T, 7)]
                else:
                    chunks = [(t0, T, 7)]
                for (a, b, last_i) in chunks:
                    nc.tensor.matmul(
                        ps_pv[:, a:b],
                        vstore[:, i, h, :],
                        wbuf[:, u, o + (a - t0):o + (b - t0)],
                        start=(i == 0), stop=(i == last_i),
                    )
            tp = tr.tile([1, T], F32, tag="tp", bufs=1)
            nc.vector.tensor_scalar_add(tp, ps_pv[64:65, :],
                                        tb_sb[0:1, h:h + 1])
            nc.vector.reciprocal_approx_fast(tp, tp)
            gb = tr.tile([64, T], F32, tag="gb", bufs=1)
            nc.gpsimd.partition_broadcast(gb, tp, channels=64)
            nc.vector.scalar_tensor_tensor(
                out=ctx[r0:r0 + 64, g, :], in0=ps_pv[0:64, :],
                scalar=vns_sb[:, h:h + 1], in1=gb,
                op0=ALU.add, op1=ALU.mult,
            )

        def emit_v_unit(tt):
            psv = pa.tile([128, 512], F32, tag="pvproj", bufs=2)
            for kt in range(KT):
                nc.tensor.matmul(
                    psv, xt[:, kt, 128 * tt:128 * (tt + 1)],
                    wv[:, kt, :], start=(kt == 0), stop=False,
                )
            nc.tensor.matmul(psv, ones_r[0:1, 0:128], bv, start=False,
                             stop=True)
            nc.vector.tensor_copy(
                vstore[:, tt, :, 0:64],
                psv.rearrange("p (h d) -> p h d", d=64),
            )

        def emit_wo_round_a():
            for mt in range(8):
                po = pa.tile([128, T], F32, tag="pvproj", bufs=2)
                for th in range(2):
                    sl = slice(512 * th, 512 * (th + 1))
                    for ci, ct in enumerate((0, 1)):
                        nc.tensor.matmul(
                            po[:, sl], wo[:, ct, mt, :], ctx[:, ct, sl],
                            start=(ci == 0), stop=(ci == 1),
                        )
                nc.vector.tensor_copy(y_acc[:, mt, :], po)

        def emit_wo_round_b():
            for mt in range(8):
                po = pa.tile([128, T], F32, tag="pvproj", bufs=2)
                for th in range(2):
                    sl = slice(512 * th, 512 * (th + 1))
                    nc.tensor.matmul(po[:, sl], wo[:, 2, mt, :],
                                     ctx[:, 2, sl], start=True, stop=True)
                nc.vector.tensor_tensor(y_acc[:, mt, :], po, y_acc[:, mt, :],
                                        op=ALU.add)

        def emit_wo_round_c():
            for mt in range(8):
                po = pa.tile([128, T], F32, tag="pvproj", bufs=2)
                for th in range(2):
                    sl = slice(512 * th, 512 * (th + 1))
                    nc.tensor.matmul(po[:, sl], wo[:, 3, mt, :],
                                     ctx[:, 3, sl], start=True, stop=True)
                ysb = tr.tile([128, T], F32, tag="ysb", bufs=1)
                nc.vector.tensor_tensor(ysb, po, y_acc[:, mt, :], op=ALU.add)
                nc.sync.dma_start(
                    out=YT.ap()[128 * mt:128 * (mt + 1), :], in_=ysb
                )

        trunc = int(os.environ.get("KTRUNC", "4"))

        def dummy_out():
            z = tr.tile([128, T], F32, tag="ysb", bufs=1)
            nc.vector.memset(z, 0.0)
            for mt in range(8):
                nc.sync.dma_start(out=YT.ap()[128 * mt:128 * (mt + 1), :],
                                  in_=z)

        # ---------- main emission ----------
        wbuf_of = {}
        exp_insts_of = {g: [] for g in range(4)}
        silu_of = {}
        pending = []  # spliceable scores+exp units of the previous wave

        def drain(n):
            for _ in range(min(n, len(pending))):
                pending.pop(0)()

        for g in range(4):
            # K projection matmuls for this group, with the previous wave's
            # leftover score tiles spliced in so the ACT exp stream stays fed
            ps_k = proj_chain(wk, g)
            drain(3)
            finish_k_phase(g, ps_k)
            drain(6)
            if g - 1 >= 0 and trunc >= 2:
                for b in range(8):
                    emit_mask(g - 1, b)
                emit_ln_silu(g - 1)
            emit_q_phase(g)
            if trunc < 2:
                continue
            # this wave's scores: first 4 units now, rest after next K phase
            wbuf_of[g] = pw.tile([128, 2, W_COLS], F16, tag="wbuf",
                                 name=f"wbuf{g}")
            units = score_tile_units(g)
            for fn in units[:4]:
                fn()
            pending = units[4:]
        if trunc < 2:
            dummy_out()

        # V projection, spliced with wave 3's remaining scores
        if trunc == 2:
            while pending:
                pending.pop(0)()
            for b in range(8):
                emit_mask(3, b)
            emit_ln_silu(3)
            dummy_out()
        vq = list(range(8)) if trunc >= 3 else []
        while pending or vq:
            if vq:
                emit_v_unit(vq.pop(0))
            if pending:
                pending.pop(0)()
            if pending:
                pending.pop(0)()
        if trunc >= 3:
            for b in range(8):
                emit_mask(3, b)
            emit_ln_silu(3)

        # attention tails + W_O rounds
        if dbg:
            nc.sync.dma_start(out=DKR.ap(), in_=krope)
            nc.sync.dma_start(out=DQR.ap(), in_=qrope)
        if trunc < 3:
            emit_head_tail = lambda *a: None
        if trunc < 4:
            emit_wo_round_a = emit_wo_round_b = emit_wo_round_c = lambda: None
        if trunc == 6:
            emit_wo_round_c = lambda: None
        emit_head_tail(0, 0)
        emit_head_tail(0, 1)
        if dbg:
            nc.sync.dma_start(out=DWB.ap(), in_=wbuf_of[0])
        emit_head_tail(1, 0)
        emit_head_tail(1, 1)
        emit_wo_round_a()
        emit_head_tail(2, 0)
        emit_head_tail(2, 1)
        emit_wo_round_b()
        if trunc >= 3:
            warm_pe(10)
        emit_head_tail(3, 0)
        emit_head_tail(3, 1)
        emit_wo_round_c()
        if trunc in (3, 6):
            dummy_out()
        if dbg:
            nc.sync.dma_start(out=DCTX.ap(), in_=ctx)

        pa.release()
        pw.release()
        tr.release()
        pk.release()
        pc.release()

    # Route exp and ln to the combined natural_log_exp_and_others ACT table
    # set (saves one table load + drain per wave): strip those functions from
    # the earlier-indexed single-function sets so the set picker can't choose
    # them. Indices (= act_func_set_id walrus remaps by) stay intact.
    import concourse.bacc as _bacc_mod
    from concourse.hw_specs import get_activation_tables as _gat

    def _gat_patched(arch):
        t = {k: set(v) for k, v in _gat(arch).items()}
        if "natural_log_exp_and_others" in t:
            for k in t:
                if k != "natural_log_exp_and_others":
                    t[k].discard(AF.Exp)
                    t[k].discard(AF.Ln)
        return t

    _bacc_mod.get_activation_tables = _gat_patched
    try:
        nc.finalize()
    finally:
        _bacc_mod.get_activation_tables = _gat
    return nc


def _host_inputs(inputs):
    """Build the 8 per-core input maps from full inputs."""
    X = np.asarray(inputs["X"], dtype=np.float32)
    W_Q = np.asarray(inputs["W_Q"], dtype=np.float32)
    b_Q = np.asarray(inputs["b_Q"], dtype=np.float32)
    W_K = np.asarray(inputs["W_K"], dtype=np.float32)
    b_K = np.asarray(inputs["b_K"], dtype=np.float32)
    W_V = np.asarray(inputs["W_V"], dtype=np.float32)
    b_V = np.asarray(inputs["b_V"], dtype=np.float32)
    sink = np.asarray(inputs["sink_scalars"], dtype=np.float32)
    v_nulls = np.asarray(inputs["v_nulls"], dtype=np.float32)
    W_O = np.asarray(inputs["W_O"], dtype=np.float32)

    XT = np.ascontiguousarray(X[0].T)  # [C, T]

    # channel permutation (evens then odds) within each head's 64 channels
    perm64 = np.concatenate([np.arange(0, 64, 2), np.arange(1, 64, 2)])
    perm512 = (np.arange(8)[:, None] * 64 + perm64[None, :]).reshape(-1)

    # RoPE tables, matching reference float32 math
    invf = (1.0 / (10000.0 ** (np.arange(0, DH, 2, dtype=np.float32) / DH))).astype(
        np.float32
    )
    freqs = np.arange(T, dtype=np.float32)[:, None] * invf[None, :]  # [T, 32]
    cos32 = np.cos(freqs).T  # [32, T]
    sin32 = np.sin(freqs).T
    cos128 = np.tile(cos32, (4, 1)).astype(np.float16)
    sin128 = np.concatenate([-sin32, sin32, -sin32, sin32], axis=0).astype(np.float16)

    # swap matrix: out[p] = q[partner(p)]; lhsT[p', p] = 1 iff p' = partner(p)
    pswap = np.zeros((128, 128), dtype=np.float16)
    for p in range(128):
        partner = p + 32 if (p % 64) < 32 else p - 32
        pswap[partner, p] = 1.0

    # key_self selectors: per-group [128, 2]: rows 0-63 -> col 0, 64-127 -> col 1
    bch = np.zeros((2, 128), dtype=np.float16)
    bch[0, 0:64] = 1.0
    bch[1, 64:128] = 1.0
    sel2 = np.zeros((128, 4, 2), dtype=np.float16)
    for g in range(4):
        sel2[0:64, g, 0] = 1.0
        sel2[64:128, g, 1] = 1.0

    in_maps = []
    for c in range(N_CORES):
        n, half = c // 2, c % 2
        qs = slice(512 * c, 512 * (c + 1))
        ks = slice(512 * half, 512 * (half + 1))
        heads = np.arange(8 * c, 8 * c + 8)
        sinks = sink[heads]  # [8]
        thr = np.tile((S * sinks).astype(np.float32)[None, :], (128, 1))
        tb = (S * (sinks + 1e-6)).astype(np.float32)[None, :]
        vn = v_nulls[n].reshape(N_HEAD, DH)  # base-head x d
        vns = np.zeros((64, 8), dtype=np.float32)
        for h in range(8):
            bh = (8 * half) + h  # base head index within branch
            vns[:, h] = S * sinks[h] * vn[bh]
        bqt = np.ascontiguousarray(
            b_Q[qs][perm512].reshape(4, 128).T
        ).astype(np.float32)
        bkt = np.ascontiguousarray(
            b_K[ks][perm512].reshape(4, 128).T
        ).astype(np.float32)
        in_maps.append(
            {
                "XT": XT.astype(np.float16),
                "WQ": np.ascontiguousarray(W_Q[:, qs][:, perm512]).astype(np.float16),
                "WK": np.ascontiguousarray(W_K[:, ks][:, perm512]).astype(np.float16),
                "WV": np.ascontiguousarray(W_V[:, ks]).astype(np.float16),
                "BQT": bqt,
                "BKT": bkt,
                "BV": np.ascontiguousarray(b_V[ks])[None, :].astype(np.float16),
                "WO": np.ascontiguousarray(0.25 * W_O[n, ks, :]),
                "COS": cos128,
                "SIN": sin128,
                "PSW": pswap,
                "SEL2": sel2,
                "BCH": bch,
                "THR": thr,
                "TB": tb,
                "VNS": vns,
                "ONES": np.ones((1, 512), dtype=np.float16),
            }
        )
    return in_maps


def kernel(**inputs) -> np.ndarray:
    from concourse.bass_utils import run_bass_kernel_spmd

    in_maps = _host_inputs(inputs)
    if _NC_CACHE[0] is None:
        _NC_CACHE[0] = _build_nc()
    nc = _NC_CACHE[0]
    trace = bool(os.environ.get("KBENCH_TRACE"))
    res = run_bass_kernel_spmd(
        nc, in_maps, core_ids=list(range(N_CORES)), trace=trace
    )
    LAST_RESULT[0] = res
    if trace and res.exec_time_ns is not None:
        print(f"HW exec time: {res.exec_time_ns} ns")

    W_O_bias = np.asarray(inputs["W_O_bias"], dtype=np.float32)
    y = np.zeros((T, D_MODEL), dtype=np.float32)
    for r in res.results:
        y += r["YT"].T
    y += W_O_bias.mean(axis=0)[None, :]
    return y[None, :, :]
